# revision 1
# baseline (speedup 1.0000x reference)
"""Self-contained Trainium2 Bass kernel for a 2-layer GAT (nn_GAT_20529943675471).

Strategy (dst-sharded graph parallel, 8 cores):
  - Host: append self-loops, shard dst-nodes across 8 cores (6250 each),
    bucket each core's dst nodes into 49 groups of 128 keyed by their
    (low-half, high-half) in-edge counts.  A group is a [128 dst x
    (L1+L2) slots] rectangle: L1 slots gather table rows < 32768, L2 slots
    rows >= 32768 (dma_gather indices are int16, so the table is addressed
    as two halves).  Pad slots point at a per-half phantom row whose
    a_src = -1e30, so exp(e - m) == 0 exactly and pads drop out of the
    softmax and weighted sum.
  - Launch A (per core): T1[row] = [x@W1 (bf16) | x@(W1@A_src) |
    x@(W1@A_dst)] (512B rows) built on PE, then per group: two dma_gather
    calls (4 SWDGE queues striped), a [128,1]-indexed indirect a_d fetch,
    logits+softmax on ACT/DVE (dst-per-partition, slots on the free dim),
    weighted sum as DVE mul + free-axis reduce, relu -> out1 shard.
  - Host: stack+transpose out1 shards -> h1T (kept in permuted order).
  - Launch B (per core): T2 = [h1@W2 | h1@b2s | h1@b2d] (256B rows,
    replicated build), same group machinery with 1 head -> out2 shard.
  - Host: un-permute shards into the full [50000, 32] float32 output.
"""

import os
import sys
import numpy as np

for _p in ("/opt/trn_rl_repo", "/opt/pypackages"):
    if _p not in sys.path:
        sys.path.insert(0, _p)

import concourse.bass as bass
import concourse.bacc as bacc
import concourse.mybir as mybir
import concourse.tile as tile
from concourse import bass_utils
from ml_dtypes import bfloat16

F32 = mybir.dt.float32
BF16 = mybir.dt.bfloat16
I32 = mybir.dt.int32
AF = mybir.ActivationFunctionType
OP = mybir.AluOpType

NEG_SLOPE = 0.2
NEG_BIG = -1.0e30


class Cfg:
    def __init__(self, N=50000, E=1600000, IN=128, HID=32, HEADS=4, NC=8,
                 mixed=True):
        assert N % NC == 0
        self.N, self.E, self.IN, self.HID, self.HEADS, self.NC = N, E, IN, HID, HEADS, NC
        self.mixed = mixed                    # h stored bf16 in gather tables
        self.P = 128
        self.PER = N // NC                    # dst nodes per core
        self.G = self.PER // 128 + 1          # groups per core (>=1 dummy row)
        self.ROWS = self.G * 128              # padded shard rows
        self.NPAD = ((N + 2 + 127) // 128) * 128   # x cols incl. phantom slots
        self.NTAB1 = self.NPAD                # T1 rows
        self.NTAB2 = self.ROWS * NC           # T2 rows (phantoms = dummy rows)
        self.D1 = IN                          # layer-1 out width (HEADS*HID)
        self.D2 = HID                         # layer-2 out width
        self.HALF = 32768                     # dma_gather int16 index range
        # table slot layouts, in f32 words; row bytes must be 256B-multiple
        # (dma_gather elem constraint)
        if mixed:
            self.SW1 = 128                    # 512B: h bf16[128] | a_s f32[4] | a_d f32[4] | pad
            self.AS1, self.AD1 = 64, 64 + HEADS
            self.SW2 = 64                     # 256B: h2 bf16[32] | a_s2 | a_d2 | pad
            self.AS2, self.AD2 = 16, 17
        else:
            self.SW1 = 192                    # 768B: h f32[128] | a_s[4] | a_d[4] | pad
            self.AS1, self.AD1 = self.D1, self.D1 + HEADS
            self.SW2 = 64                     # 256B: h2 f32[32] | a_s2 | a_d2 | pad
            self.AS2, self.AD2 = self.D2, self.D2 + 1
        self.HDT = BF16 if mixed else F32
        self.np_h = bfloat16 if mixed else np.float32


# ----------------------------------------------------------------------------
# Host-side preprocessing
# ----------------------------------------------------------------------------

def build_schedule(cfg, edge_index):
    """Degree-bucketed group schedule with per-half slot rectangles.

    Per group g each dst row has L1[g] low-half slots then L2[g] high-half
    slots (padded with the respective phantom row; all indices valid).
    Layer tables:
      T1 rows: tblrow1(v) = v + (v >= HALF)  (row HALF-1 = phantom-low,
               row N+1 -> cfg.PH1_HI = phantom-high)
      T2 rows: permuted id (k*ROWS + pos); phantoms are dummy shard rows.
    """
    N, NC, PER, G, P = cfg.N, cfg.NC, cfg.PER, cfg.G, cfg.P
    HALF = cfg.HALF
    src = np.asarray(edge_index[0], dtype=np.int64)
    dst = np.asarray(edge_index[1], dtype=np.int64)
    loop = np.arange(N, dtype=np.int64)
    src = np.concatenate([loop, src])
    dst = np.concatenate([loop, dst])
    order = np.argsort(dst, kind="stable")
    src_s = src[order].astype(np.int64)
    dst_s = dst[order]
    deg = np.bincount(dst_s, minlength=N)
    starts = np.zeros(N + 1, dtype=np.int64)
    np.cumsum(deg, out=starts[1:])

    shift1_pre = cfg.N + 2 > HALF
    # per-dst low-half src counts, layer 1 (x-row space)
    src_row1 = src_s + (src_s >= (HALF - 1)) if shift1_pre else src_s
    lo1 = np.bincount(dst_s[src_row1 < HALF], minlength=N)
    hi1 = deg - lo1
    # perm1: bucket by (lo1, hi1) so both rectangle maxima are tight
    perm1 = np.full((NC, cfg.ROWS), -1, dtype=np.int64)
    for k in range(NC):
        sl = slice(k * PER, (k + 1) * PER)
        key = lo1[sl] * 1000 + hi1[sl]
        perm1[k, :PER] = np.argsort(-key, kind="stable")

    # layer-2 remap of GLOBAL node -> T2 row (depends on perm1)
    g2p = np.zeros(N, dtype=np.int64)
    for k in range(NC):
        v = perm1[k] >= 0
        g2p[k * PER + perm1[k, v]] = k * cfg.ROWS + np.nonzero(v)[0]

    # perm2: same trick in T2-row space
    lo2 = np.bincount(dst_s[g2p[src_s] < HALF], minlength=N)
    hi2 = deg - lo2
    perm2 = np.full((NC, cfg.ROWS), -1, dtype=np.int64)
    for k in range(NC):
        sl = slice(k * PER, (k + 1) * PER)
        key = lo2[sl] * 1000 + hi2[sl]
        perm2[k, :PER] = np.argsort(-key, kind="stable")
    perms = [perm1, perm2]

    shift1 = shift1_pre                # row shift only when table spans halves
    L1 = np.zeros((2, G), np.int64)    # [layer, g] low-rect width
    L2 = np.zeros((2, G), np.int64)
    cell = {}                          # (layer, k, g) -> (lo_list, hi_list)
    for layer in (0, 1):
        pm = perms[layer]
        for k in range(NC):
            for g in range(G):
                nodes = pm[k, g * P:(g + 1) * P]
                valid = nodes >= 0
                gl = np.where(valid, nodes + k * PER, 0)
                dg = np.where(valid, deg[gl], 0).astype(np.int64)
                tot = int(dg.sum())
                rows = np.repeat(np.arange(P), dg)
                within = np.arange(tot) - np.repeat(np.cumsum(dg) - dg, dg)
                epos = np.repeat(starts[gl], dg) + within
                s = src_s[epos]
                if layer == 0:
                    rid = s + (s >= (HALF - 1)) if shift1 else s.copy()
                else:
                    rid = g2p[s]
                lo_mask = rid < HALF
                lob = [rid[(rows == p) & lo_mask] for p in range(P)]
                hib = [rid[(rows == p) & ~lo_mask] for p in range(P)]
                cell[(layer, k, g)] = (lob, hib)
                L1[layer, g] = max(L1[layer, g],
                                   max((len(x) for x in lob), default=0))
                L2[layer, g] = max(L2[layer, g],
                                   max((len(x) for x in hib), default=0))
    L1 = np.maximum(L1, 1)
    Ltot = L1 + L2
    offs = np.zeros((2, G + 1), np.int64)
    offs[0, 1:] = np.cumsum(Ltot[0] * P)
    offs[1, 1:] = np.cumsum(Ltot[1] * P)
    olo = np.zeros((2, G + 1), np.int64)
    olo[0, 1:] = np.cumsum(L1[0] * P)
    olo[1, 1:] = np.cumsum(L1[1] * P)
    ohi = np.zeros((2, G + 1), np.int64)
    ohi[0, 1:] = np.cumsum(L2[0] * P)
    ohi[1, 1:] = np.cumsum(L2[1] * P)

    ph_lo = [HALF - 1 if shift1 else cfg.N,            # phantom rows (global)
             cfg.ROWS - 1]
    ph_hi = [cfg.N + 1 if shift1 else (HALF - 1 if shift1 else cfg.N),
             NC * cfg.ROWS - 1]
    cfg.PH1_LO, cfg.PH1_HI = ph_lo[0], ph_hi[0]
    cfg.PH2_LO, cfg.PH2_HI = ph_lo[1], ph_hi[1]

    def wrap16(vals):
        v = vals.reshape(-1, 16).T.astype(np.int16)
        return np.tile(v, (8, 1))

    planes = {}
    idx32 = {}                                         # int32 per-slot rows (tests)
    dstrows = {}
    for layer in (0, 1):
        plo = np.empty((NC, 128, int(olo[layer, -1]) // 16), np.int16)
        phi = np.empty((NC, 128, int(ohi[layer, -1]) // 16), np.int16)
        ix = np.empty((NC, int(offs[layer, -1])), np.int64)
        dr = np.empty((NC, G * P), np.int64)
        for k in range(NC):
            for g in range(G):
                lob, hib = cell[(layer, k, g)]
                w1, w2 = int(L1[layer, g]), int(L2[layer, g])
                blo = np.full((P, w1), ph_lo[layer], np.int64)
                bhi = np.full((P, w2), ph_hi[layer], np.int64)
                for p in range(P):
                    blo[p, :len(lob[p])] = lob[p]
                    bhi[p, :len(hib[p])] = hib[p]
                o = int(olo[layer, g])
                plo[k, :, o // 16:o // 16 + 8 * w1] = wrap16(blo.T.ravel())
                o = int(ohi[layer, g])
                phi[k, :, o // 16:o // 16 + 8 * w2] = wrap16(
                    (bhi - HALF).T.ravel())
                o = int(offs[layer, g])
                ix[k, o:o + P * (w1 + w2)] = np.concatenate(
                    [blo, bhi], axis=1).ravel()
                # per-dst table row for the a_d gather
                nodes = perms[layer][k, g * P:(g + 1) * P]
                valid = nodes >= 0
                gl = np.where(valid, nodes + k * PER, 0)
                if layer == 0:
                    rr = gl + (gl >= (HALF - 1)) if shift1 else gl.copy()
                    rr = np.where(valid, rr, ph_lo[0])
                else:
                    rr = np.where(valid, g2p[gl], ph_lo[1])
                dr[k, g * P:(g + 1) * P] = rr
        planes[(layer, "lo")] = plo
        planes[(layer, "hi")] = phi
        idx32[layer] = ix
        dstrows[layer] = dr

    return dict(L1=L1, L2=L2, Ltot=Ltot, offs=offs, olo=olo, ohi=ohi,
                perm=perm2, shift1=shift1,
                TOT1=int(offs[0, -1]), TOT2=int(offs[1, -1]),
                i1lo=planes[(0, "lo")], i1hi=planes[(0, "hi")],
                i2lo=planes[(1, "lo")], i2hi=planes[(1, "hi")],
                idx1=idx32[0], idx2=idx32[1],
                dr1=dstrows[0].astype(np.int32),
                dr2=dstrows[1].astype(np.int32))


def host_params(cfg, W1, as1, ad1, W2, as2, ad2):
    """Fold attention vectors through the linear maps.

    W1cat = [W1 | W1@A_src | W1@A_dst]  (IN x (D1 + 2H))
    W2cat = [W2 | W2@as2^T | W2@ad2^T]  (D1 x (D2 + 2))
    """
    H, C = cfg.HEADS, cfg.HID
    A_s = np.zeros((cfg.D1, H), np.float32)
    A_d = np.zeros((cfg.D1, H), np.float32)
    for h in range(H):
        A_s[h * C:(h + 1) * C, h] = as1[h]
        A_d[h * C:(h + 1) * C, h] = ad1[h]
    W1cat = np.concatenate([W1, W1 @ A_s, W1 @ A_d], axis=1).astype(np.float32)
    W2cat = np.concatenate([W2, W2 @ as2.reshape(-1, 1),
                            W2 @ ad2.reshape(-1, 1)], axis=1).astype(np.float32)
    return W1cat, W2cat


# ----------------------------------------------------------------------------
# Device programs
# ----------------------------------------------------------------------------

def _table_views(cfg, T, layer):
    """(h_view_fn, as_col, ad_col, sw) for table T."""
    sw = cfg.SW1 if layer == 1 else cfg.SW2
    return sw


def _emit_table_build(cfg, nc, tc, ctx, xT, wcat, T, nblocks, strips, sw, asc,
                      d_out, wn):
    """T[rows, :] = [xT.T @ Wcat] with h cast to HDT, a_s/a_d kept f32.

    xT: [128, cols] AP (DRAM), wcat: SBUF [128, d_out + extra], T: DRAM handle
    strips: list of (col0, ncols) DMA-load strips covering nblocks*128 cols.
    """
    extra = (2 * cfg.HEADS) if wn == 1 else 2
    xp = ctx.enter_context(tc.tile_pool(name=f"xp{wn}", bufs=3))
    pp = ctx.enter_context(tc.tile_pool(name=f"pp{wn}", bufs=4, space="PSUM"))
    hp = ctx.enter_context(tc.tile_pool(name=f"hp{wn}", bufs=4))
    apids = ctx.enter_context(tc.tile_pool(name=f"ap{wn}", bufs=4))
    Tf = T.bitcast(F32) if cfg.mixed else T   # f32 word view of table rows
    i = 0
    for (c0, ncols) in strips:
        xt = xp.tile([128, ncols], xT.dtype)
        nc.sync.dma_start(out=xt[:], in_=xT[:, c0:c0 + ncols])
        for m in range(ncols // 128):
            r0 = c0 + m * 128
            ps = pp.tile([128, d_out + extra], F32, tag="ps")
            nc.tensor.matmul(ps[:], lhsT=xt[:, m * 128:(m + 1) * 128],
                             rhs=wcat[:], start=True, stop=True)
            # both copies on ONE engine per tile (PE wait-slot budget is tiny)
            ht = hp.tile([128, d_out], cfg.HDT, tag="ht")
            at = apids.tile([128, extra], F32, tag="at")
            if i % 2 == 0:
                nc.vector.tensor_copy(out=ht[:], in_=ps[:, 0:d_out])
                nc.vector.tensor_copy(out=at[:], in_=ps[:, d_out:d_out + extra])
            else:
                nc.scalar.copy(out=ht[:], in_=ps[:, 0:d_out])
                nc.scalar.copy(out=at[:], in_=ps[:, d_out:d_out + extra])
            nc.sync.dma_start(out=T[r0:r0 + 128, 0:d_out], in_=ht[:])
            nc.sync.dma_start(out=Tf[r0:r0 + 128, asc:asc + extra], in_=at[:])
            i += 1


def _emit_phantom(cfg, nc, tc, ctx, T, sw, asc, nheads, row):
    Tf = T.bitcast(F32) if cfg.mixed else T
    cp = ctx.enter_context(tc.tile_pool(name=f"ph{row}", bufs=1))
    ph = cp.tile([1, sw], F32)
    nc.vector.memset(ph[:], 0.0)
    nc.vector.memset(ph[:, asc:asc + nheads], NEG_BIG)
    nc.sync.dma_start(out=Tf[row:row + 1, :], in_=ph[:])


def _emit_groups(cfg, nc, tc, ctx, sched, layer, T, idx_d, out_d, brep):
    """Edge-processing groups for one layer."""
    P = cfg.P
    H = cfg.HEADS if layer == 1 else 1
    C = cfg.HID
    D = H * C
    sw = cfg.SW1 if layer == 1 else cfg.SW2          # f32 words per slot
    asc = cfg.AS1 if layer == 1 else cfg.AS2
    adc = cfg.AD1 if layer == 1 else cfg.AD2
    elem = sw if not cfg.mixed else 2 * sw           # elems in table dtype
    ly = layer - 1
    Tap = T.ap()
    Tf_full = T.bitcast(F32).ap() if cfg.mixed else Tap
    Tlo = Tap
    Thi = Tap[cfg.HALF:, :] if T.shape[0] > cfg.HALF else Tap
    idx_lo, idx_hi, dst_d = idx_d                    # DRAM planes + dst rows

    ip = ctx.enter_context(tc.tile_pool(name=f"ip{layer}", bufs=5))
    gp = ctx.enter_context(tc.tile_pool(name=f"gp{layer}", bufs=3))
    ep = ctx.enter_context(tc.tile_pool(name=f"ep{layer}", bufs=2))
    xp = ctx.enter_context(tc.tile_pool(name=f"exp{layer}", bufs=2))
    sp = ctx.enter_context(tc.tile_pool(name=f"sp{layer}", bufs=3))
    ptp = ctx.enter_context(tc.tile_pool(name=f"pt{layer}", bufs=1))
    op_ = ctx.enter_context(tc.tile_pool(name=f"op{layer}", bufs=3))

    for g in range(cfg.G):
        w1 = int(sched["L1"][ly, g])
        w2 = int(sched["L2"][ly, g])
        Lg = w1 + w2
        olo = int(sched["olo"][ly, g]) // 16
        ohi = int(sched["ohi"][ly, g]) // 16
        gt = gp.tile([P, Lg * elem], cfg.HDT, tag="gt")
        gv = gt[:].rearrange("p (l e) -> p l e", e=elem)
        qn = (2 * g) % 4
        ilo = ip.tile([P, 8 * w1], mybir.dt.int16, tag="ilo")
        nc.sync.dma_start(out=ilo[:], in_=idx_lo.ap()[:, olo:olo + 8 * w1])
        nc.gpsimd.dma_gather(out_ap=gv[:, 0:w1, :], in_ap=Tlo, idxs_ap=ilo[:],
                             num_idxs=P * w1, num_idxs_reg=P * w1,
                             elem_size=elem, single_packet=False,
                             queue_num=qn)
        if w2:
            ihi = ip.tile([P, 8 * w2], mybir.dt.int16, tag="ihi")
            nc.sync.dma_start(out=ihi[:],
                              in_=idx_hi.ap()[:, ohi:ohi + 8 * w2])
            nc.gpsimd.dma_gather(out_ap=gv[:, w1:Lg, :], in_ap=Thi,
                                 idxs_ap=ihi[:], num_idxs=P * w2,
                                 num_idxs_reg=P * w2, elem_size=elem,
                                 single_packet=False, queue_num=(qn + 1) % 4)
        # per-dst a_d via row-per-partition indirect gather
        idd = ip.tile([P, 1], I32, tag="idd")
        nc.sync.dma_start(out=idd[:],
                          in_=dst_d.ap()[g * P:(g + 1) * P].rearrange(
                              "(p o) -> p o", o=1))
        adt = sp.tile([P, H], F32, tag="adt")
        nc.gpsimd.indirect_dma_start(
            out=adt[:], out_offset=None, in_=Tf_full,
            in_offset=bass.IndirectOffsetOnAxis(ap=idd[:], axis=0),
            element_offset=adc)

        gf = gt[:].bitcast(F32) if cfg.mixed else gt[:]
        gl = gf.rearrange("p (l w) -> p l w", w=sw)
        # e[p, h*L + l] = lrelu(a_s[p,l,h] + a_d[p,h])
        # lrelu(v) = max(v, 0.2*v) since 0 < alpha < 1
        e0 = ep.tile([P, H * Lg], F32, tag="e0")
        et = ep.tile([P, H * Lg], F32, tag="et")
        for h in range(H):
            nc.scalar.activation(
                e0[:, h * Lg:(h + 1) * Lg], gl[:, :, asc + h],
                AF.Identity, bias=adt[:, h:h + 1], scale=1.0)
        nc.vector.tensor_scalar(out=et[:], in0=e0[:], scalar1=NEG_SLOPE,
                                scalar2=None, op0=OP.mult)
        nc.vector.tensor_tensor(out=et[:], in0=et[:], in1=e0[:], op=OP.max)
        # m = -max_l e ; ex = exp(e + m) ; den = sum_l ex
        mt = sp.tile([P, H], F32, tag="mt")
        den = sp.tile([P, H], F32, tag="den")
        ext = xp.tile([P, H * Lg], F32, tag="ext")
        for h in range(H):
            nc.vector.tensor_reduce(
                mt[:, h:h + 1], et[:, h * Lg:(h + 1) * Lg],
                axis=mybir.AxisListType.X, op=OP.max, negate=True)
            nc.scalar.activation(
                ext[:, h * Lg:(h + 1) * Lg], et[:, h * Lg:(h + 1) * Lg],
                AF.Exp, bias=mt[:, h:h + 1], accum_out=den[:, h:h + 1])
        rec = sp.tile([P, H], F32, tag="rec")
        nc.vector.reciprocal(rec[:], den[:])

        # products: pt[p, l, h, c] = h_tab[p, l, h, c] * ex[p, h, l]
        hv = (gt[:].rearrange("p (l e) -> p l e", e=elem)
              [:, :, 0:D].rearrange("p l (h c) -> p l h c", c=C))
        exv = (ext[:].rearrange("p (h l) -> p h l", h=H)
               .transpose([0, 2, 1]).unsqueeze(3).to_broadcast([P, Lg, H, C]))
        pt = ptp.tile([P, Lg * D], cfg.HDT, tag="ptt")
        ptv = pt[:].rearrange("p (l h c) -> p l h c", h=H, c=C)
        nc.vector.tensor_tensor(out=ptv, in0=hv, in1=exv, op=OP.mult)
        # sum over slots l, keep (h, c): in-place pairwise tree on the bf16
        # product tile (dense step-1 adds run at DVE 2x; tensor_reduce is 1x).
        # Odd n leaves slot `half` untouched and carries it to the next level.
        pl = pt[:].rearrange("p (l d) -> p l d", d=D)
        st = op_.tile([P, D], F32, tag="st")
        n = Lg
        while n > 1:
            half = n // 2
            if n == 2:
                nc.vector.tensor_tensor(out=st[:], in0=pl[:, 0, :],
                                        in1=pl[:, 1, :], op=OP.add)
            else:
                nc.vector.tensor_tensor(out=pl[:, 0:half, :],
                                        in0=pl[:, 0:half, :],
                                        in1=pl[:, n - half:n, :], op=OP.add)
            n -= half
        if Lg == 1:
            nc.vector.tensor_copy(out=st[:], in_=pl[:, 0, :])
        # scale by 1/den, add bias
        ot = op_.tile([P, D], F32, tag="ot")
        if H > 1:
            rv = rec[:].unsqueeze(2).to_broadcast([P, H, C])
            nc.vector.tensor_tensor(
                out=ot[:].rearrange("p (h c) -> p h c", c=C),
                in0=st[:].rearrange("p (h c) -> p h c", c=C),
                in1=rv, op=OP.mult)
        else:
            nc.vector.tensor_scalar(out=ot[:], in0=st[:],
                                    scalar1=rec[:, 0:1], scalar2=None,
                                    op0=OP.mult)
        nc.vector.tensor_tensor(out=ot[:], in0=ot[:], in1=brep[:], op=OP.add)
        if layer == 1:
            rt = op_.tile([P, D], cfg.HDT, tag="rt")
            nc.scalar.activation(rt[:], ot[:], AF.Relu)
            nc.sync.dma_start(out=out_d[g * P:(g + 1) * P, :], in_=rt[:])
        else:
            nc.sync.dma_start(out=out_d[g * P:(g + 1) * P, :], in_=ot[:])


def build_launchA(cfg, sched):
    nc = bacc.Bacc("TRN2", target_bir_lowering=False, num_devices=cfg.NC,
                   debug=False, enable_partition_id=False,
                   num_swdge_queues=4, dynamic_dma_scratch_size=65536)
    xT = nc.dram_tensor("xT", [128, cfg.NPAD], F32, kind="ExternalInput")
    w1 = nc.dram_tensor("W1cat", [128, cfg.D1 + 2 * cfg.HEADS], F32,
                        kind="ExternalInput")
    i1lo = nc.dram_tensor("i1lo", [128, int(sched["olo"][0, -1]) // 16],
                          mybir.dt.int16, kind="ExternalInput")
    i1hi = nc.dram_tensor("i1hi", [128, max(int(sched["ohi"][0, -1]), 16) // 16],
                          mybir.dt.int16, kind="ExternalInput")
    dr1 = nc.dram_tensor("dr1", [cfg.G * 128], I32, kind="ExternalInput")
    b1r = nc.dram_tensor("b1rep", [128, cfg.D1], F32, kind="ExternalInput")
    out1 = nc.dram_tensor("out1", [cfg.ROWS, cfg.D1], cfg.HDT,
                          kind="ExternalOutput")
    elem1 = cfg.SW1 * (2 if cfg.mixed else 1)
    T1 = nc.dram_tensor("T1", [cfg.NTAB1, elem1], cfg.HDT, kind="Internal")

    from contextlib import ExitStack
    with tile.TileContext(nc) as tc, ExitStack() as ctx:
        cp = ctx.enter_context(tc.tile_pool(name="constA", bufs=1))
        w1s = cp.tile([128, cfg.D1 + 2 * cfg.HEADS], F32)
        nc.sync.dma_start(out=w1s[:], in_=w1.ap())
        b1s = cp.tile([128, cfg.D1], F32)
        nc.sync.dma_start(out=b1s[:], in_=b1r.ap())
        strips, c0 = [], 0
        while c0 < cfg.NPAD:
            w = min(512, cfg.NPAD - c0)
            strips.append((c0, w))
            c0 += w
        _emit_table_build(cfg, nc, tc, ctx, xT.ap(), w1s[:], T1, None, strips,
                          cfg.SW1, cfg.AS1, cfg.D1, 1)
        _emit_phantom(cfg, nc, tc, ctx, T1, cfg.SW1, cfg.AS1, cfg.HEADS,
                      cfg.PH1_LO)
        if cfg.PH1_HI != cfg.PH1_LO:
            _emit_phantom(cfg, nc, tc, ctx, T1, cfg.SW1, cfg.AS1, cfg.HEADS,
                          cfg.PH1_HI)
        # the gather's dynamic reads of T1 are not byte-range tracked
        # against the table-write DMAs -> explicit barrier
        tc.strict_bb_all_engine_barrier()
        _emit_groups(cfg, nc, tc, ctx, sched, 1, T1, (i1lo, i1hi, dr1), out1,
                     b1s[:])
    nc.compile()
    return nc


def build_launchB(cfg, sched):
    nc = bacc.Bacc("TRN2", target_bir_lowering=False, num_devices=cfg.NC,
                   debug=False, enable_partition_id=False,
                   num_swdge_queues=4, dynamic_dma_scratch_size=65536)
    h1T = nc.dram_tensor("h1T", [cfg.NC * 128, cfg.ROWS], cfg.HDT,
                         kind="ExternalInput")
    w2 = nc.dram_tensor("W2cat", [128, cfg.D2 + 2], cfg.HDT,
                        kind="ExternalInput")
    i2lo = nc.dram_tensor("i2lo", [128, int(sched["olo"][1, -1]) // 16],
                          mybir.dt.int16, kind="ExternalInput")
    i2hi = nc.dram_tensor("i2hi", [128, max(int(sched["ohi"][1, -1]), 16) // 16],
                          mybir.dt.int16, kind="ExternalInput")
    dr2 = nc.dram_tensor("dr2", [cfg.G * 128], I32, kind="ExternalInput")
    b2r = nc.dram_tensor("b2rep", [128, cfg.D2], F32, kind="ExternalInput")
    out2 = nc.dram_tensor("out2", [cfg.ROWS, cfg.D2], F32,
                          kind="ExternalOutput")
    elem2 = cfg.SW2 * (2 if cfg.mixed else 1)
    T2 = nc.dram_tensor("T2", [cfg.NTAB2, elem2], cfg.HDT, kind="Internal")

    from contextlib import ExitStack
    with tile.TileContext(nc) as tc, ExitStack() as ctx:
        cp = ctx.enter_context(tc.tile_pool(name="constB", bufs=1))
        w2s = cp.tile([128, cfg.D2 + 2], cfg.HDT)
        nc.sync.dma_start(out=w2s[:], in_=w2.ap())
        b2s = cp.tile([128, cfg.D2], F32)
        nc.sync.dma_start(out=b2s[:], in_=b2r.ap())

        sp = ctx.enter_context(tc.tile_pool(name="strip", bufs=2))
        pp = ctx.enter_context(tc.tile_pool(name="ppB", bufs=4, space="PSUM"))
        hp = ctx.enter_context(tc.tile_pool(name="hpB", bufs=4))
        apd = ctx.enter_context(tc.tile_pool(name="apB", bufs=4))
        T2f = T2.bitcast(F32) if cfg.mixed else T2
        i = 0
        for c in range(cfg.NC):
            strip = sp.tile([128, cfg.ROWS], cfg.HDT, tag="strip")
            nc.sync.dma_start(out=strip[:],
                              in_=h1T.ap()[c * 128:(c + 1) * 128, :])
            for b in range(cfg.G):
                r0 = c * cfg.ROWS + b * 128
                ps = pp.tile([128, cfg.D2 + 2], F32, tag="ps")
                nc.tensor.matmul(ps[:], lhsT=strip[:, b * 128:(b + 1) * 128],
                                 rhs=w2s[:], start=True, stop=True)
                ht = hp.tile([128, cfg.D2], cfg.HDT, tag="ht")
                at = apd.tile([128, 2], F32, tag="at")
                if i % 2 == 0:
                    nc.vector.tensor_copy(out=ht[:], in_=ps[:, 0:cfg.D2])
                    nc.vector.tensor_copy(out=at[:], in_=ps[:, cfg.D2:cfg.D2 + 2])
                else:
                    nc.scalar.copy(out=ht[:], in_=ps[:, 0:cfg.D2])
                    nc.scalar.copy(out=at[:], in_=ps[:, cfg.D2:cfg.D2 + 2])
                nc.sync.dma_start(out=T2[r0:r0 + 128, 0:cfg.D2], in_=ht[:])
                nc.sync.dma_start(out=T2f[r0:r0 + 128, cfg.AS2:cfg.AS2 + 2],
                                  in_=at[:])
                i += 1
        _emit_phantom(cfg, nc, tc, ctx, T2, cfg.SW2, cfg.AS2, 1, cfg.PH2_LO)
        _emit_phantom(cfg, nc, tc, ctx, T2, cfg.SW2, cfg.AS2, 1, cfg.PH2_HI)
        tc.strict_bb_all_engine_barrier()
        _emit_groups(cfg, nc, tc, ctx, sched, 2, T2, (i2lo, i2hi, dr2), out2,
                     b2s[:])
    nc.compile()
    return nc


# ----------------------------------------------------------------------------
# Orchestration
# ----------------------------------------------------------------------------

LAST_PROFILE = []


def _hi_plane(cfg, sched, key, k):
    arr = sched[key][k]
    if arr.shape[1] == 0:
        return np.zeros((128, 1), np.int16)
    return arr


def _prep_inputs(cfg, sched, x, W1cat, W2cat, b1, b2):
    xp = np.zeros((cfg.NPAD, cfg.IN), np.float32)
    v = np.arange(cfg.N)
    rows = v + (v >= (cfg.HALF - 1)) if sched["shift1"] else v
    xp[rows] = x
    xT = np.ascontiguousarray(xp.T)
    b1rep = np.broadcast_to(b1.astype(np.float32), (128, cfg.D1)).copy()
    b2rep = np.broadcast_to(b2.astype(np.float32), (128, cfg.D2)).copy()
    inA = []
    for k in range(cfg.NC):
        inA.append({"xT": xT, "W1cat": W1cat, "i1lo": sched["i1lo"][k],
                    "i1hi": _hi_plane(cfg, sched, "i1hi", k),
                    "dr1": sched["dr1"][k], "b1rep": b1rep})
    return inA, b2rep


_TEST_CFG = None     # tests may override problem size


def kernel(x, edge_index, W1, as1, ad1, b1, W2, as2, ad2, b2):
    global LAST_PROFILE
    LAST_PROFILE = []
    cfg = _TEST_CFG or Cfg()
    x = np.asarray(x, np.float32)
    W1 = np.asarray(W1, np.float32)
    W2 = np.asarray(W2, np.float32)
    sched = build_schedule(cfg, np.asarray(edge_index))
    W1cat, W2cat = host_params(cfg, W1, np.asarray(as1, np.float32),
                               np.asarray(ad1, np.float32), W2,
                               np.asarray(as2, np.float32),
                               np.asarray(ad2, np.float32))
    inA, b2rep = _prep_inputs(cfg, sched, x, W1cat, W2cat,
                              np.asarray(b1, np.float32),
                              np.asarray(b2, np.float32))

    ncA = build_launchA(cfg, sched)
    resA = bass_utils.run_bass_kernel_spmd(
        ncA, inA, core_ids=list(range(cfg.NC)))
    LAST_PROFILE.append(resA)

    # h1T[c*128:(c+1)*128, :] = out1_c.T  (stays permuted)
    h1T = np.empty((cfg.NC * 128, cfg.ROWS), cfg.np_h)
    for k in range(cfg.NC):
        h1T[k * 128:(k + 1) * 128, :] = resA.results[k]["out1"].T
    w2h = W2cat.astype(cfg.np_h)
    inB = [{"h1T": h1T, "W2cat": w2h, "i2lo": sched["i2lo"][k],
            "i2hi": _hi_plane(cfg, sched, "i2hi", k),
            "dr2": sched["dr2"][k], "b2rep": b2rep} for k in range(cfg.NC)]
    ncB = build_launchB(cfg, sched)
    resB = bass_utils.run_bass_kernel_spmd(
        ncB, inB, core_ids=list(range(cfg.NC)))
    LAST_PROFILE.append(resB)

    out = np.empty((cfg.N, cfg.D2), np.float32)
    for k in range(cfg.NC):
        p = sched["perm"][k]
        v = p >= 0
        out[k * cfg.PER + p[v]] = resB.results[k]["out2"][np.nonzero(v)[0]]
    return out

